# revision 2
# baseline (speedup 1.0000x reference)
"""Trainium2 kernel for nn_BlockLinear: gather -> per-block GEMM -> scatter-add.

Key insight: the whole op is linear in x, so gather/einsum/scatter fold into a
single dense GEMM  out[t, o] = sum_k x[t, k] * Wfull[k, o] + bias[o]  where
Wfull[k, o] = sum_{n,i,j} [input_indices[n,i]==k][output_indices[n,j]==o] * W[n,j,i].

Wfull is built on host (bincount scatter-add, exact fp64 accumulation), then the
GEMM runs on 8 NeuronCores, sharded 2D: 4 token groups x 2 out-feature groups.
Matmuls run in bf16 (same PE rate as fp32r, half the HBM/DMA traffic, so the
x-streaming warmup phase is no longer DMA-starved); accumulation is fp32 in
PSUM and the bias add + output stay full fp32.
"""

import numpy as np
import ml_dtypes
import concourse.bacc as bacc
import concourse.mybir as mybir
import concourse.tile as tile
from concourse.bass_utils import run_bass_kernel_spmd

# problem shapes (hardcoded per contract)
B, S = 2, 2048
IN_FEATURES = 4096
OUT_FEATURES = 4096
NTOKENS = B * S                  # 4096

NCORES = 8
TG, OG = 4, 2                    # token groups x out-feature groups
T = NTOKENS // TG                # 1024 tokens per core
O = OUT_FEATURES // OG           # 2048 out features per core
P = 128
KT = IN_FEATURES // P            # 32 contraction tiles
OT = O // P                      # 16 out-feature tiles per core
NTOK = 512                       # moving free dim per matmul
TB = T // NTOK                   # 2 token blocks per core

BF16 = mybir.dt.bfloat16
F32 = mybir.dt.float32

# knobs for test.py
TRACE = False
LAST_RESULTS = None

WCHUNK = 8        # k-tiles per W DMA (2KB/partition contiguous)
WBUFS = 16        # W chunk pool bufs (4 o-groups in flight)
NDUMMY = 9        # PE HAM warmup matmuls (~3.8us at 1.2 GHz)


def build_nc(repeats: int = 1):
    nc = bacc.Bacc()
    # xT slabs: [k][128, TB*NTOK] bf16
    xw = nc.dram_tensor("xw", [KT, P, TB * NTOK], BF16, kind="ExternalInput")
    # W chunked [o][kc][128, WCHUNK, 128] bf16 (partition-contiguous chunks)
    KC = KT // WCHUNK
    wrest = nc.dram_tensor(
        "wrest", [OT, KC, P, WCHUNK, P], BF16, kind="ExternalInput"
    )
    # bias in o-partition layout: [128, OT]
    bo = nc.dram_tensor("bo", [P, OT], F32, kind="ExternalInput")
    out = nc.dram_tensor("out", [OT, TB, P, NTOK], F32, kind="ExternalOutput")

    NWARM = 4  # o-groups processed k-major while the xT stream arrives

    with tile.TileContext(nc) as tc:
        with (
            tc.tile_pool(name="xw_sb", bufs=1) as xw_sb,
            tc.tile_pool(name="w_sb", bufs=WBUFS) as w_sb,
            tc.tile_pool(name="o_sb", bufs=6) as o_sb,
            tc.tile_pool(name="ps", bufs=8, space="PSUM") as ps,
        ):
            bo_t = xw_sb.tile([P, OT], F32, tag="bo")

            # PE HAM warmup: dummy matmuls on memset data fill the dead time
            # while the first DMAs land, so real matmuls start at 2.4 GHz
            dummy_sb = xw_sb.tile([P, NTOK], BF16, tag="dummy")
            nc.vector.memset(dummy_sb.bitcast(F32), 0.0)
            ps_d = ps.tile([P, NTOK], F32, tag="ps", name="ps_dummy")
            for _ in range(NDUMMY):
                nc.tensor.matmul(
                    ps_d, dummy_sb[:, :P], dummy_sb, start=True, stop=True
                )

            wts = {}

            def load_w(o, rep):
                for kc in range(KC):
                    wt = w_sb.tile(
                        [P, WCHUNK, P], BF16, tag="wt", name=f"wt_{rep}_{o}_{kc}"
                    )
                    # alternate issue queues to halve SP issue bursts
                    eng = nc.sync if kc % 2 == 0 else nc.scalar
                    eng.dma_start(out=wt, in_=wrest[o, kc])
                    wts[o, kc] = wt

            # xT slabs issue k-major on the (otherwise idle) ACT queue while W
            # chunks issue on SP, interleaved in warmup consumption order
            xw_t = {}
            for kc in range(KC):
                for o in range(NWARM):
                    load_w_chunk = w_sb.tile(
                        [P, WCHUNK, P], BF16, tag="wt", name=f"wt_0_{o}_{kc}"
                    )
                    nc.sync.dma_start(out=load_w_chunk, in_=wrest[o, kc])
                    wts[o, kc] = load_w_chunk
                    # interleave xw issues between W issues so neither stream
                    # blocks the other's first arrivals
                    k = kc * WCHUNK + o
                    if o < WCHUNK:
                        t = xw_sb.tile([P, TB * NTOK], BF16, tag=f"xw_{k}")
                        nc.scalar.dma_start(out=t, in_=xw[k])
                        xw_t[k] = t
                for k in range(kc * WCHUNK, (kc + 1) * WCHUNK):
                    if k not in xw_t:
                        t = xw_sb.tile([P, TB * NTOK], BF16, tag=f"xw_{k}")
                        nc.scalar.dma_start(out=t, in_=xw[k])
                        xw_t[k] = t
                if kc == 0:
                    # bias load is only needed by the drains, ~60us later;
                    # keep its issue slot off the critical path
                    nc.sync.dma_start(out=bo_t, in_=bo[:, :])

            def drain(o, tb, psum):
                o_t = o_sb.tile([P, NTOK], F32, tag="ot", name=f"ot_{o}_{tb}")
                # psum -> sbuf with per-partition bias add; alternate engines
                # so consecutive drains run in parallel
                if (o * TB + tb) % 2 == 0:
                    nc.scalar.add(o_t, psum, bo_t[:, o : o + 1])
                else:
                    nc.vector.tensor_scalar_add(o_t, psum, bo_t[:, o : o + 1])
                nc.scalar.dma_start(out=out[o, tb, :, :], in_=o_t)

            def mm_group(o, rep):
                psums = {
                    tb: ps.tile([P, NTOK], F32, tag="ps", name=f"ps_{rep}_{o}_{tb}")
                    for tb in range(TB)
                }
                for k in range(KT):
                    lhsT = wts[o, k // WCHUNK][:, k % WCHUNK]
                    for tb in range(TB):
                        nc.tensor.matmul(
                            psums[tb],
                            lhsT,
                            xw_t[k][:, tb * NTOK : (tb + 1) * NTOK],
                            start=(k == 0),
                            stop=(k == KT - 1),
                        )
                for tb in range(TB):
                    drain(o, tb, psums[tb])

            for _rep in range(repeats):
                if _rep == 0:
                    # warmup phase: k-major over NWARM o-groups x TB token
                    # blocks (all 8 psum banks) -> 8 matmuls per arriving
                    # xT k-slab, keeping the PE busy while xT streams in
                    psums = {
                        (o, tb): ps.tile(
                            [P, NTOK], F32, tag="ps", name=f"psw_{o}_{tb}"
                        )
                        for o in range(NWARM)
                        for tb in range(TB)
                    }
                    for k in range(KT - WCHUNK):
                        for o in range(NWARM):
                            lhsT = wts[o, k // WCHUNK][:, k % WCHUNK]
                            for tb in range(TB):
                                nc.tensor.matmul(
                                    psums[o, tb],
                                    lhsT,
                                    xw_t[k][:, tb * NTOK : (tb + 1) * NTOK],
                                    start=(k == 0),
                                    stop=False,
                                )
                    # last k-window o-major with immediate drains, so psum
                    # banks free one o-group at a time and the steady phase
                    # starts while the rest of the warmup finishes
                    for o in range(NWARM):
                        for k in range(KT - WCHUNK, KT):
                            lhsT = wts[o, k // WCHUNK][:, k % WCHUNK]
                            for tb in range(TB):
                                nc.tensor.matmul(
                                    psums[o, tb],
                                    lhsT,
                                    xw_t[k][:, tb * NTOK : (tb + 1) * NTOK],
                                    start=False,
                                    stop=(k == KT - 1),
                                )
                        for tb in range(TB):
                            drain(o, tb, psums[o, tb])
                    o_start = NWARM
                else:
                    o_start = 0
                for o in range(o_start, OT):
                    load_w(o, _rep)
                    mm_group(o, _rep)
    nc.finalize()
    return nc


_NC = None


def _get_nc():
    global _NC
    if _NC is None:
        _NC = build_nc()
    return _NC


def _build_wfull(weights, input_indices, output_indices):
    """Wfull[k, o] = sum over blocks/dups of weights[n, j, i]."""
    ii = np.asarray(input_indices).astype(np.int64)     # [NBLK, BI]
    oi = np.asarray(output_indices).astype(np.int64)    # [NBLK, BO]
    w = np.asarray(weights, dtype=np.float64)           # [NBLK, BO, BI]
    flat = (ii[:, :, None] * OUT_FEATURES + oi[:, None, :]).ravel()  # [n, i, j]
    vals = np.ascontiguousarray(np.swapaxes(w, 1, 2)).ravel()        # [n, i, j]
    wfull = np.bincount(flat, weights=vals, minlength=IN_FEATURES * OUT_FEATURES)
    return wfull.reshape(IN_FEATURES, OUT_FEATURES)


def prepare_in_maps(x, weights, bias, input_indices, output_indices):
    x = np.asarray(x, dtype=np.float32)
    bias = np.asarray(bias, dtype=np.float32)

    KC = KT // WCHUNK
    wfull = _build_wfull(weights, input_indices, output_indices).astype(
        ml_dtypes.bfloat16
    )
    xr = x.reshape(NTOKENS, IN_FEATURES).astype(ml_dtypes.bfloat16)

    in_maps = []
    for c in range(NCORES):
        tg, og = divmod(c, OG)
        xT = np.ascontiguousarray(xr[tg * T : (tg + 1) * T, :].T)   # [K, T]
        xwm = np.ascontiguousarray(xT.reshape(KT, P, T))
        # [K, O/2] -> [OT, KC, P(k), WCHUNK, P(o)] partition-contiguous chunks
        wr = np.ascontiguousarray(
            wfull[:, og * O : (og + 1) * O]
            .reshape(KC, WCHUNK, P, OT, P)
            .transpose(3, 0, 2, 1, 4)
        )
        # bias in o-partition layout [128, OT]; full fp32 (added exactly on ACT)
        bo = np.ascontiguousarray(
            bias[og * O : (og + 1) * O].reshape(OT, P).T
        )
        in_maps.append({"xw": xwm, "wrest": wr, "bo": bo})
    return in_maps


def assemble_output(core_outs):
    full = np.empty((NTOKENS, OUT_FEATURES), np.float32)
    for c in range(NCORES):
        tg, og = divmod(c, OG)
        o4 = np.asarray(core_outs[c])                    # [OT, TB, P, NTOK]
        blk = o4.transpose(1, 3, 0, 2).reshape(T, O)     # [t, o]
        full[tg * T : (tg + 1) * T, og * O : (og + 1) * O] = blk
    return full.reshape(B, S, OUT_FEATURES)


def kernel(x, weights, bias, input_indices, output_indices):
    global LAST_RESULTS
    in_maps = prepare_in_maps(x, weights, bias, input_indices, output_indices)
    nc = _get_nc()
    res = run_bass_kernel_spmd(nc, in_maps, list(range(NCORES)))
    LAST_RESULTS = res
    return assemble_output([res.results[c]["out"] for c in range(NCORES)])
